# revision 6
# baseline (speedup 1.0000x reference)
"""Trainium2 Bass kernel for nn_DecentralController (gnn_message_passing).

Sharding: data-parallel over batch B=16 across 8 cores (2 samples = 40
images per core, params replicated; no collectives needed). Conv layers
run as tap-stacked bf16 matmuls (contraction = 3*C_in via shifted stacks,
the x-shift applied as a free-dim offset on the rhs), BN+bias+LeakyReLU
fused into the ScalarE Lrelu activation at PSUM-evict, maxpool as TT-max
on the VectorE, image-ganged col-tiling (tile_position) to fill the PE
output dim, then a 144-chunk compress matmul and a per-sample graph
filter + MLP tail in fp32.
"""
import os
import sys
from contextlib import ExitStack

sys.path.insert(0, "/opt/trn_rl_repo")

import numpy as np
import ml_dtypes

import concourse.bass as bass
import concourse.mybir as mybir
from concourse import bacc, tile
from concourse.bass_utils import run_bass_kernel_spmd

BF16 = mybir.dt.bfloat16
F32 = mybir.dt.float32
AF = mybir.ActivationFunctionType
ALU = mybir.AluOpType
LEAK = 0.01

B, N = 16, 20
NCORES = 8
SPC = B // NCORES          # samples per core
IPC = SPC * N              # images per core (40)
NGANG = IPC // 4
DEBUG = bool(int(os.environ.get("KBASS_DEBUG", "0")))

nbf = lambda a: np.ascontiguousarray(np.asarray(a, np.float32)).astype(ml_dtypes.bfloat16)
nf32 = lambda a: np.ascontiguousarray(np.asarray(a, np.float64)).astype(np.float32)


# ---------------------------------------------------------------- host prep
def _prep_weights(conv_ws, conv_bs, bn_gammas, bn_betas, bn_means, bn_vars,
                  comp_w, comp_b, gf_h, gf_b, act_ws, act_bs):
    wf, bf = [], []
    for l in range(5):
        w = np.asarray(conv_ws[l], np.float64)
        inv = np.asarray(bn_gammas[l], np.float64) / np.sqrt(
            np.asarray(bn_vars[l], np.float64) + 1e-5)
        wf.append(w * inv[:, None, None, None])
        bf.append(np.asarray(conv_bs[l], np.float64) * inv
                  + np.asarray(bn_betas[l], np.float64)
                  - np.asarray(bn_means[l], np.float64) * inv)

    d = {}
    # L0 s2d weights: stack partition = 4*(sy+1) + (py*2+px);
    # output partition = (qy*2+qx)*32 + o.
    w0 = np.zeros((3, 12, 128), np.float64)
    for sxi, sx in enumerate((-1, 0, 1)):
        for byi, sy in enumerate((-1, 0, 1)):
            for py in range(2):
                for px in range(2):
                    for qy in range(2):
                        for qx in range(2):
                            ky, kx = 2 * sy + py - qy, 2 * sx + px - qx
                            if -1 <= ky <= 1 and -1 <= kx <= 1:
                                g = qy * 2 + qx
                                w0[sxi, 4 * byi + py * 2 + px,
                                   g * 32:(g + 1) * 32] = wf[0][:, 0, ky + 1, kx + 1]
    d["w0s"] = nbf(w0.transpose(1, 0, 2).reshape(12, 384))

    def _stack3(w):  # [O, C, 3, 3] -> [3dx, 3dy*C, O], dy blocks (0,-1,+1)
        O, C = w.shape[0], w.shape[1]
        out = np.zeros((3, 3 * C, O), np.float64)
        for dxi, dx in enumerate((-1, 0, 1)):
            for bi, dy in enumerate((0, -1, 1)):
                out[dxi, bi * C:(bi + 1) * C, :] = w[:, :, dy + 1, dx + 1].T
        return out

    d["w1s"] = nbf(_stack3(wf[1]).transpose(1, 0, 2).reshape(96, 96))
    d["w2s"] = nbf(_stack3(wf[2]).transpose(1, 0, 2).reshape(96, 192))
    s3 = _stack3(wf[3])
    d["w3a"] = nbf(s3[:, :128].transpose(1, 0, 2).reshape(128, 192))
    d["w3b"] = nbf(s3[:, 128:].transpose(1, 0, 2).reshape(64, 192))
    s4 = _stack3(wf[4])
    d["w4a"] = nbf(s4[:, :128].transpose(1, 0, 2).reshape(128, 384))
    d["w4b"] = nbf(s4[:, 128:].transpose(1, 0, 2).reshape(64, 384))

    cw = np.asarray(comp_w, np.float64).reshape(128, 144, 128)
    d["compw"] = nbf(cw.transpose(1, 0, 2))              # [144, 128c, 128f]
    d["compb"] = nf32(np.asarray(comp_b)[None, :])       # [1, 128]
    d["ones1"] = nf32(np.ones((1, IPC)))

    bias = np.zeros((6, 128, 1), np.float64)
    bias[0, :, 0] = np.tile(bf[0], 4)
    bias[1, :, 0] = np.tile(bf[1], 4)
    bias[2, :, 0] = np.tile(bf[2], 2)
    bias[3, :, 0] = np.tile(bf[3], 2)
    bias[4, :, 0] = bf[4]
    bias[5, :, 0] = np.asarray(gf_b, np.float64)
    d["biases"] = nf32(bias[:, :, 0].T)  # [128, 6]

    gfh = np.asarray(gf_h, np.float64)                   # [F, 3, F]
    d["hkT"] = nf32(np.concatenate([gfh[:, k, :].T for k in range(3)], axis=1))  # [128, 384]
    a0 = np.asarray(act_ws[0], np.float64)
    d["w1a"], d["w1b"] = nf32(a0[:128]), nf32(a0[128:])
    d["w2m"] = nf32(act_ws[1])
    d["w3m"] = nf32(act_ws[2])
    mb = np.zeros((2, 128, 1), np.float64)
    mb[0, :, 0] = np.asarray(act_bs[0], np.float64)
    mb[1, :, 0] = np.asarray(act_bs[1], np.float64)
    d["mbias"] = nf32(mb[:, :, 0].T)  # [128, 2]
    d["b3m"] = nf32(np.asarray(act_bs[2])[:, None])      # [2, 1]
    d["id20"] = nf32(np.eye(20))
    return d


def _prep_xs2d(x):  # [B, N, 100, 100] -> [B*N, 4, 54, 52] zero-padded bf16
    BN = x.shape[0] * x.shape[1]
    xr = np.asarray(x, np.float32).reshape(BN, 50, 2, 50, 2)
    out = np.zeros((BN, 4, 54, 52), np.float32)
    for py in range(2):
        for px in range(2):
            out[:, py * 2 + px, 2:52, 1:51] = xr[:, :, py, :, px]
    return out.astype(ml_dtypes.bfloat16)


# ---------------------------------------------------------------- device program
def _emit(nc, st):
    P = {}
    def par(name, shape, dt=BF16, out=False):
        P[name] = nc.declare_dram_parameter(name, list(shape), dt, isOutput=out)

    par("xs", (IPC, 4, 54, 52))
    par("w0s", (12, 384)); par("w1s", (96, 96)); par("w2s", (96, 192))
    par("w3a", (128, 192)); par("w3b", (64, 192))
    par("w4a", (128, 384)); par("w4b", (64, 384))
    par("compw", (144, 128, 128))
    par("compb", (1, 128), F32); par("ones1", (1, IPC), F32)
    par("biases", (128, 6), F32)
    par("Ssb", (20, SPC * 20), F32); par("hkT", (128, 384), F32)
    par("w1a", (128, 128), F32); par("w1b", (20, 128), F32)
    par("w2m", (128, 128), F32); par("w3m", (128, 2), F32)
    par("extras", (20, SPC * 20), F32); par("id20", (20, 20), F32)
    par("mbias", (128, 2), F32); par("b3m", (2, 1), F32)
    par("out", (SPC, 20, 2), F32, out=True)
    if DEBUG:
        par("dbg_feat", (IPC, 128), F32, out=True)
        par("dbg_pool0", (96, 2704), BF16, out=True)
        par("dbg_y1", (128, 2704), BF16, out=True)
        par("dbg_pool2", (128, 625), BF16, out=True)
        par("dbg_hall", (128, 144), F32, out=True)

    tc = st.enter_context(tile.TileContext(nc))
    pers = st.enter_context(tc.tile_pool(name="pers", bufs=1))
    sb = st.enter_context(tc.tile_pool(name="sb", bufs=2))
    sbt = st.enter_context(tc.tile_pool(name="sbt", bufs=6))
    cps = st.enter_context(tc.tile_pool(name="cps", bufs=5, space="PSUM"))
    tps = st.enter_context(tc.tile_pool(name="tps", bufs=2, space="PSUM"))
    cwp = st.enter_context(tc.tile_pool(name="cwp", bufs=8))

    def load(name, shape, dt=BF16, src=None, tag=None):
        t = pers.tile(list(shape), dt, tag=tag or name)
        nc.sync.dma_start(t[:], (src if src is not None else P[name])[:])
        return t

    w0s = load("w0s", (12, 384))
    w1s = load("w1s", (96, 96))
    w2s = load("w2s", (96, 192))
    w3a = load("w3a", (128, 192))
    w3b = load("w3b", (64, 192))
    w4a = load("w4a", (128, 384))
    w4b = load("w4b", (64, 384))
    biases = load("biases", (128, 6), F32)
    compb = load("compb", (1, 128), F32)
    ones1 = load("ones1", (1, IPC), F32)
    hall = pers.tile([128, IPC * 144], BF16, tag="hall")

    WS = {
        0: lambda dx: w0s[:, dx * 128:(dx + 1) * 128],
        1: lambda dx: w1s[:, dx * 32:(dx + 1) * 32],
        2: lambda dx: w2s[:, dx * 64:(dx + 1) * 64],
        "3a": lambda dx: w3a[:, dx * 64:(dx + 1) * 64],
        "3b": lambda dx: w3b[:, dx * 64:(dx + 1) * 64],
        "4a": lambda dx: w4a[:, dx * 128:(dx + 1) * 128],
        "4b": lambda dx: w4b[:, dx * 128:(dx + 1) * 128],
    }
    BIA = lambda i: biases[:, i:i + 1]

    def mkstacks(tag, n, p, f):
        ts = [pers.tile([p, f], BF16, tag=f"{tag}{i}", name=f"{tag}{i}")
              for i in range(n)]
        for t in ts:
            nc.gpsimd.memset(t[:], 0.0)
        return ts

    st1 = mkstacks("st1", 8, 96, 2704)     # slot = (gang%2)*4 + im%4
    st2 = mkstacks("st2", 4, 96, 2704)     # slot = ((im//2)%2)*2 + im%2
    st3a = mkstacks("st3a", 4, 128, 729)   # slot = ((im//2)%2)*2 + im%2
    st3b = mkstacks("st3b", 4, 64, 729)
    st4a = mkstacks("st4a", 2, 128, 729)   # slot = im%2
    st4b = mkstacks("st4b", 2, 64, 729)

    CH52 = [(r, min(8, 51 - r)) for r in range(1, 51, 8)]
    CH27 = [(1, 18), (19, 7)]

    def i3(t, p0, pn, r0, nr, c0, ncol, rs):
        return t[:].rearrange("p (r c) -> p r c", c=rs)[p0:p0 + pn,
                                                        r0:r0 + nr, c0:c0 + ncol]

    # ------------------------------------------------ L0 + pool0 -> st1
    def do_L0(im):
        stk = sb.tile([12, 52 * 52], BF16, tag="l0stk")
        for byi, sy in enumerate((-1, 0, 1)):
            nc.sync.dma_start(
                stk[:].rearrange("p (r c) -> p r c", c=52)[4 * byi:4 * byi + 4],
                P["xs"][im, :, 1 + sy:53 + sy, :])
        y = sb.tile([128, 2704], BF16, tag="l0y")
        for r, nr in CH52:
            pt = cps.tile([128, 416], F32, tag="cps")
            for dxi in range(3):
                nc.tensor.matmul(pt[:, :nr * 52], WS[0](dxi),
                                 stk[:, r * 52 + dxi - 1:(r + nr) * 52 + dxi - 1],
                                 start=(dxi == 0), stop=(dxi == 2))
            nc.scalar.activation(y[:, r * 52:(r + nr) * 52], pt[:, :nr * 52],
                                 AF.Lrelu, bias=BIA(0), alpha=LEAK)
        z1 = sbt.tile([32, 2704], BF16, tag="l0tmp")
        z2 = sbt.tile([32, 2704], BF16, tag="l0tmp")
        z3 = sbt.tile([32, 2704], BF16, tag="l0tmp")
        nc.sync.dma_start(z1[:, 52:2652], y[32:64, 52:2652])
        nc.sync.dma_start(z2[:, 52:2652], y[64:96, 52:2652])
        nc.sync.dma_start(z3[:, 52:2652], y[96:128, 52:2652])
        t1 = sbt.tile([32, 2704], BF16, tag="l0tmp")
        t2 = sbt.tile([32, 2704], BF16, tag="l0tmp")
        nc.vector.tensor_tensor(t1[:, 52:2652], y[0:32, 52:2652],
                                z1[:, 52:2652], op=ALU.max)
        nc.vector.tensor_tensor(t2[:, 52:2652], z2[:, 52:2652],
                                z3[:, 52:2652], op=ALU.max)
        dst = st1[((im // 4) % 2) * 4 + im % 4]
        nc.vector.tensor_tensor(i3(dst, 0, 32, 1, 50, 1, 50, 52),
                                i3(t1, 0, 32, 1, 50, 1, 50, 52),
                                i3(t2, 0, 32, 1, 50, 1, 50, 52), op=ALU.max)
        nc.sync.dma_start(i3(dst, 32, 32, 2, 50, 1, 50, 52),
                          i3(dst, 0, 32, 1, 50, 1, 50, 52))
        nc.sync.dma_start(i3(dst, 64, 32, 0, 50, 1, 50, 52),
                          i3(dst, 0, 32, 1, 50, 1, 50, 52))
        if DEBUG and im == 0:
            nc.sync.dma_start(P["dbg_pool0"][:], dst[:])

    # ------------------------------------------------ L1 (gang of 4) -> st2
    def do_L1(g):
        stks = [st1[(g % 2) * 4 + i] for i in range(4)]
        y = sb.tile([128, 2704], BF16, tag="l1y")
        for r, nr in CH52:
            pt = cps.tile([128, 416], F32, tag="cps")
            for i in range(4):
                for dxi in range(3):
                    nc.tensor.matmul(pt[32 * i:32 * i + 32, :nr * 52], WS[1](dxi),
                                     stks[i][:, r * 52 + dxi - 1:(r + nr) * 52 + dxi - 1],
                                     start=(dxi == 0), stop=(dxi == 2),
                                     tile_position=(0, 32 * i))
            nc.scalar.activation(y[:, r * 52:(r + nr) * 52], pt[:, :nr * 52],
                                 AF.Lrelu, bias=BIA(1), alpha=LEAK)
        if DEBUG and g == 0:
            nc.sync.dma_start(P["dbg_y1"][:], y[:])
        for i in range(4):
            im = 4 * g + i
            dst = st2[((im // 2) % 2) * 2 + im % 2]
            src = i3(y, 32 * i, 32, 1, 50, 1, 50, 52)
            nc.sync.dma_start(i3(dst, 0, 32, 1, 50, 1, 50, 52), src)
            nc.sync.dma_start(i3(dst, 32, 32, 2, 50, 1, 50, 52), src)
            nc.sync.dma_start(i3(dst, 64, 32, 0, 50, 1, 50, 52), src)

    # ------------------------------------------------ L2 (gang of 2) + pool -> st3
    def do_L2(g):
        stks = [st2[(g % 2) * 2 + i] for i in range(2)]
        y = sb.tile([128, 2704], BF16, tag="l2y")
        for r, nr in CH52:
            pt = cps.tile([128, 416], F32, tag="cps")
            for i in range(2):
                for dxi in range(3):
                    nc.tensor.matmul(pt[64 * i:64 * i + 64, :nr * 52], WS[2](dxi),
                                     stks[i][:, r * 52 + dxi - 1:(r + nr) * 52 + dxi - 1],
                                     start=(dxi == 0), stop=(dxi == 2),
                                     tile_position=(0, 64 * i))
            nc.scalar.activation(y[:, r * 52:(r + nr) * 52], pt[:, :nr * 52],
                                 AF.Lrelu, bias=BIA(2), alpha=LEAK)
        p1 = sb.tile([128, 1300], BF16, tag="l2p1")
        yv = y[:].rearrange("p (r c) -> p r c", c=52)
        nc.vector.tensor_tensor(p1[:].rearrange("p (r c) -> p r c", c=52),
                                yv[:, 1:51:2, :], yv[:, 2:52:2, :], op=ALU.max)
        p2 = sb.tile([128, 625], BF16, tag="l2p2")
        p1v = p1[:].rearrange("p (r c) -> p r c", c=52)
        nc.vector.tensor_tensor(p2[:].rearrange("p (r c) -> p r c", c=25),
                                p1v[:, :, 1:51:2], p1v[:, :, 2:52:2], op=ALU.max)
        if DEBUG and g == 0:
            nc.sync.dma_start(P["dbg_pool2"][:], p2[:])
        for i in range(2):
            im = 2 * g + i
            sl = ((im // 2) % 2) * 2 + im % 2
            src = p2[:].rearrange("p (r c) -> p r c", c=25)[64 * i:64 * i + 64]
            nc.sync.dma_start(i3(st3a[sl], 0, 64, 1, 25, 1, 25, 27), src)
            nc.sync.dma_start(i3(st3a[sl], 64, 64, 2, 25, 1, 25, 27), src)
            nc.sync.dma_start(i3(st3b[sl], 0, 64, 0, 25, 1, 25, 27), src)

    # ------------------------------------------------ L3 (gang of 2) -> st4
    def do_L3(g):
        y = sb.tile([128, 729], BF16, tag="l3y")
        for r, nr in CH27:
            pt = cps.tile([128, 486], F32, tag="cps")
            for i in range(2):
                sl = (g % 2) * 2 + i
                for dxi in range(3):
                    nc.tensor.matmul(pt[64 * i:64 * i + 64, :nr * 27], WS["3a"](dxi),
                                     st3a[sl][:, r * 27 + dxi - 1:(r + nr) * 27 + dxi - 1],
                                     start=(dxi == 0), stop=False,
                                     tile_position=(0, 64 * i))
                for dxi in range(3):
                    nc.tensor.matmul(pt[64 * i:64 * i + 64, :nr * 27], WS["3b"](dxi),
                                     st3b[sl][:, r * 27 + dxi - 1:(r + nr) * 27 + dxi - 1],
                                     start=False, stop=(dxi == 2),
                                     tile_position=(0, 64 * i))
            nc.scalar.activation(y[:, r * 27:(r + nr) * 27], pt[:, :nr * 27],
                                 AF.Lrelu, bias=BIA(3), alpha=LEAK)
        for i in range(2):
            im = 2 * g + i
            src = i3(y, 64 * i, 64, 1, 25, 1, 25, 27)
            nc.sync.dma_start(i3(st4a[im % 2], 0, 64, 1, 25, 1, 25, 27), src)
            nc.sync.dma_start(i3(st4a[im % 2], 64, 64, 2, 25, 1, 25, 27), src)
            nc.sync.dma_start(i3(st4b[im % 2], 0, 64, 0, 25, 1, 25, 27), src)

    # ------------------------------------------------ L4 + pool -> hall
    def do_L4(im):
        y = sb.tile([128, 729], BF16, tag="l4y")
        for r, nr in CH27:
            pt = cps.tile([128, 486], F32, tag="cps")
            for dxi in range(3):
                nc.tensor.matmul(pt[:, :nr * 27], WS["4a"](dxi),
                                 st4a[im % 2][:, r * 27 + dxi - 1:(r + nr) * 27 + dxi - 1],
                                 start=(dxi == 0), stop=False)
            for dxi in range(3):
                nc.tensor.matmul(pt[:, :nr * 27], WS["4b"](dxi),
                                 st4b[im % 2][:, r * 27 + dxi - 1:(r + nr) * 27 + dxi - 1],
                                 start=False, stop=(dxi == 2))
            nc.scalar.activation(y[:, r * 27:(r + nr) * 27], pt[:, :nr * 27],
                                 AF.Lrelu, bias=BIA(4), alpha=LEAK)
        p1 = sb.tile([128, 324], BF16, tag="l4p1")
        yv = y[:].rearrange("p (r c) -> p r c", c=27)
        nc.vector.tensor_tensor(p1[:].rearrange("p (r c) -> p r c", c=27),
                                yv[:, 1:25:2, :], yv[:, 2:26:2, :], op=ALU.max)
        p1v = p1[:].rearrange("p (r c) -> p r c", c=27)
        nc.vector.tensor_tensor(
            hall[:, im * 144:(im + 1) * 144].rearrange("p (r c) -> p r c", c=12),
            p1v[:, :, 1:25:2], p1v[:, :, 2:26:2], op=ALU.max)
        if DEBUG and im == 0:
            dh = sb.tile([128, 144], F32, tag="dbgh")
            nc.vector.tensor_copy(dh[:], hall[:, 0:144])
            nc.sync.dma_start(P["dbg_hall"][:], dh[:])

    # pipelined emission
    for g in range(NGANG):
        for i in range(4):
            do_L0(4 * g + i)
        do_L1(g)
        for h in (2 * g, 2 * g + 1):
            do_L2(h)
            do_L3(h)
        for i in range(4):
            do_L4(4 * g + i)

    # ------------------------------------------------ compress
    fp = tps.tile([IPC, 128], F32, tag="tps")
    for p in range(144):
        cw = cwp.tile([128, 128], BF16, tag="cwt")
        nc.sync.dma_start(cw[:], P["compw"][p])
        nc.tensor.matmul(fp[:], hall[:, p:p + 144 * (IPC - 1) + 1:144], cw[:],
                         start=(p == 0), stop=False)
    nc.tensor.matmul(fp[:], ones1[:], compb[:], start=False, stop=True)
    fsb = pers.tile([IPC, 128], F32, tag="fsb")
    nc.scalar.activation(fsb[:], fp[:], AF.Lrelu, alpha=LEAK)
    if DEBUG:
        nc.sync.dma_start(P["dbg_feat"][:], fsb[:])

    # ------------------------------------------------ tail (fp32)
    id20 = load("id20", (20, 20), F32)
    hkT = load("hkT", (128, 384), F32)
    w1a = load("w1a", (128, 128), F32)
    w1b = load("w1b", (20, 128), F32)
    w2m = load("w2m", (128, 128), F32)
    w3m = load("w3m", (128, 2), F32)
    mbias = load("mbias", (128, 2), F32)
    b3m = load("b3m", (2, 1), F32)
    Ss = load("Ssb", (20, SPC * 20), F32)
    exs = load("extras", (20, SPC * 20), F32)

    for s in range(SPC):
        w0 = pers.tile([20, 128], F32, tag=f"w0_{s}")
        if s == 0:
            nc.vector.tensor_copy(w0[:], fsb[0:20, :])
        else:
            nc.sync.dma_start(w0[:], fsb[20 * s:20 * s + 20, :])
        Sb = Ss[:, 20 * s:20 * s + 20]

        z0p = tps.tile([128, 20], F32, tag="tps")
        nc.tensor.transpose(z0p[:], w0[:], id20[:])
        z0 = pers.tile([128, 20], F32, tag=f"z0_{s}")
        nc.vector.tensor_copy(z0[:], z0p[:])

        w1p = tps.tile([20, 128], F32, tag="tps")
        nc.tensor.matmul(w1p[:], Sb, w0[:], start=True, stop=True)
        w1t = pers.tile([20, 128], F32, tag=f"w1t_{s}")
        nc.vector.tensor_copy(w1t[:], w1p[:])

        z1p = tps.tile([128, 20], F32, tag="tps")
        nc.tensor.matmul(z1p[:], w0[:], Sb, start=True, stop=True)
        z1 = pers.tile([128, 20], F32, tag=f"z1_{s}")
        nc.vector.tensor_copy(z1[:], z1p[:])

        z2p = tps.tile([128, 20], F32, tag="tps")
        nc.tensor.matmul(z2p[:], w1t[:], Sb, start=True, stop=True)
        z2 = pers.tile([128, 20], F32, tag=f"z2_{s}")
        nc.vector.tensor_copy(z2[:], z2p[:])

        yp = tps.tile([128, 20], F32, tag="tps")
        for k, zk in enumerate((z0, z1, z2)):
            nc.tensor.matmul(yp[:], hkT[:, 128 * k:128 * (k + 1)], zk[:],
                             start=(k == 0), stop=(k == 2))
        sfh = pers.tile([128, 20], F32, tag=f"sfh_{s}")
        nc.scalar.activation(sfh[:], yp[:], AF.Lrelu, bias=BIA(5), alpha=LEAK)

        a1p = tps.tile([128, 20], F32, tag="tps")
        nc.tensor.matmul(a1p[:], w1a[:], sfh[:], start=True, stop=False)
        nc.tensor.matmul(a1p[:], w1b[:], exs[:, 20 * s:20 * s + 20],
                         start=False, stop=True)
        a1 = pers.tile([128, 20], F32, tag=f"a1_{s}")
        nc.scalar.activation(a1[:], a1p[:], AF.Lrelu, bias=mbias[:, 0:1], alpha=LEAK)

        a2p = tps.tile([128, 20], F32, tag="tps")
        nc.tensor.matmul(a2p[:], w2m[:], a1[:], start=True, stop=True)
        a2 = pers.tile([128, 20], F32, tag=f"a2_{s}")
        nc.scalar.activation(a2[:], a2p[:], AF.Lrelu, bias=mbias[:, 1:2], alpha=LEAK)

        a3p = tps.tile([2, 20], F32, tag="tps")
        nc.tensor.matmul(a3p[:], w3m[:], a2[:], start=True, stop=True)
        a3 = pers.tile([2, 20], F32, tag=f"a3_{s}")
        nc.scalar.activation(a3[:], a3p[:], AF.Identity, bias=b3m[:])
        nc.sync.dma_start(P["out"][s].rearrange("n c -> c n"), a3[:])


_CACHED = {}


def _get_module():
    if "nc" not in _CACHED:
        nc = bacc.Bacc("TRN2", target_bir_lowering=False)
        with ExitStack() as st:
            _emit(nc, st)
        nc.finalize()
        _CACHED["nc"] = nc
    return _CACHED["nc"]


def kernel(x, S, refs, alphas, conv_ws, conv_bs, bn_gammas, bn_betas, bn_means,
           bn_vars, comp_w, comp_b, gf_h, gf_b, act_ws, act_bs):
    wd = _prep_weights(conv_ws, conv_bs, bn_gammas, bn_betas, bn_means, bn_vars,
                       comp_w, comp_b, gf_h, gf_b, act_ws, act_bs)
    xs = _prep_xs2d(x)
    refs = np.asarray(refs, np.float32)
    alphas = np.asarray(alphas, np.float32)
    S = np.asarray(S, np.float32)
    ex = np.concatenate([np.repeat(refs.transpose(0, 2, 1), 10, axis=1),
                         np.repeat(alphas.transpose(0, 2, 1), 10, axis=1)], axis=1)

    in_maps = []
    for c in range(NCORES):
        m = dict(wd)
        m["xs"] = xs[c * IPC:(c + 1) * IPC]
        m["Ssb"] = nf32(np.concatenate(list(S[c * SPC:(c + 1) * SPC]), axis=1))
        m["extras"] = nf32(np.concatenate(list(ex[c * SPC:(c + 1) * SPC]), axis=1))
        in_maps.append(m)

    nc = _get_module()
    res = run_bass_kernel_spmd(nc, in_maps, list(range(NCORES)))
    kernel.last_results = res.results
    out = np.concatenate([res.results[c]["out"] for c in range(NCORES)], axis=0)
    return out.reshape(B, N, 2).astype(np.float32)


# revision 7
# speedup vs baseline: 1.0339x; 1.0339x over previous
"""Trainium2 Bass kernel for nn_DecentralController (gnn_message_passing).

Sharding: data-parallel over batch B=16 across 8 cores (2 samples = 40
images per core, params replicated; no collectives needed). Conv layers
run as tap-stacked bf16 matmuls (contraction = 3*C_in via shifted stacks,
the x-shift applied as a free-dim offset on the rhs), BN+bias+LeakyReLU
fused into the ScalarE Lrelu activation at PSUM-evict, maxpool as TT-max
on the VectorE, image-ganged col-tiling (tile_position) to fill the PE
output dim, then a 144-chunk compress matmul and a per-sample graph
filter + MLP tail in fp32.
"""
import os
import sys
from contextlib import ExitStack

sys.path.insert(0, "/opt/trn_rl_repo")

import numpy as np
import ml_dtypes

import concourse.bass as bass
import concourse.mybir as mybir
from concourse import bacc, tile
from concourse.bass_utils import run_bass_kernel_spmd

BF16 = mybir.dt.bfloat16
F32 = mybir.dt.float32
AF = mybir.ActivationFunctionType
ALU = mybir.AluOpType
LEAK = 0.01

B, N = 16, 20
NCORES = 8
SPC = B // NCORES          # samples per core
IPC = SPC * N              # images per core (40)
NGANG = IPC // 4
DEBUG = bool(int(os.environ.get("KBASS_DEBUG", "0")))

nbf = lambda a: np.ascontiguousarray(np.asarray(a, np.float32)).astype(ml_dtypes.bfloat16)
nf32 = lambda a: np.ascontiguousarray(np.asarray(a, np.float64)).astype(np.float32)


# ---------------------------------------------------------------- host prep
def _prep_weights(conv_ws, conv_bs, bn_gammas, bn_betas, bn_means, bn_vars,
                  comp_w, comp_b, gf_h, gf_b, act_ws, act_bs):
    wf, bf = [], []
    for l in range(5):
        w = np.asarray(conv_ws[l], np.float64)
        inv = np.asarray(bn_gammas[l], np.float64) / np.sqrt(
            np.asarray(bn_vars[l], np.float64) + 1e-5)
        wf.append(w * inv[:, None, None, None])
        bf.append(np.asarray(conv_bs[l], np.float64) * inv
                  + np.asarray(bn_betas[l], np.float64)
                  - np.asarray(bn_means[l], np.float64) * inv)

    d = {}
    # L0 s2d weights: stack partition = 4*(sy+1) + (py*2+px);
    # output partition = (qy*2+qx)*32 + o.
    w0 = np.zeros((3, 12, 128), np.float64)
    for sxi, sx in enumerate((-1, 0, 1)):
        for byi, sy in enumerate((-1, 0, 1)):
            for py in range(2):
                for px in range(2):
                    for qy in range(2):
                        for qx in range(2):
                            ky, kx = 2 * sy + py - qy, 2 * sx + px - qx
                            if -1 <= ky <= 1 and -1 <= kx <= 1:
                                g = qy * 2 + qx
                                w0[sxi, 4 * byi + py * 2 + px,
                                   g * 32:(g + 1) * 32] = wf[0][:, 0, ky + 1, kx + 1]
    d["w0s"] = nbf(w0.transpose(1, 0, 2).reshape(12, 384))

    def _stack3(w):  # [O, C, 3, 3] -> [3dx, 3dy*C, O], dy blocks (0,-1,+1)
        O, C = w.shape[0], w.shape[1]
        out = np.zeros((3, 3 * C, O), np.float64)
        for dxi, dx in enumerate((-1, 0, 1)):
            for bi, dy in enumerate((0, -1, 1)):
                out[dxi, bi * C:(bi + 1) * C, :] = w[:, :, dy + 1, dx + 1].T
        return out

    d["w1s"] = nbf(_stack3(wf[1]).transpose(1, 0, 2).reshape(96, 96))
    d["w2s"] = nbf(_stack3(wf[2]).transpose(1, 0, 2).reshape(96, 192))
    s3 = _stack3(wf[3])
    d["w3a"] = nbf(s3[:, :128].transpose(1, 0, 2).reshape(128, 192))
    d["w3b"] = nbf(s3[:, 128:].transpose(1, 0, 2).reshape(64, 192))
    s4 = _stack3(wf[4])
    d["w4a"] = nbf(s4[:, :128].transpose(1, 0, 2).reshape(128, 384))
    d["w4b"] = nbf(s4[:, 128:].transpose(1, 0, 2).reshape(64, 384))

    cw = np.asarray(comp_w, np.float64).reshape(128, 144, 128)
    d["compw"] = nbf(cw.transpose(1, 0, 2))              # [144, 128c, 128f]
    d["compb"] = nf32(np.asarray(comp_b)[None, :])       # [1, 128]
    d["ones1"] = nf32(np.ones((1, IPC)))

    bias = np.zeros((6, 128, 1), np.float64)
    bias[0, :, 0] = np.tile(bf[0], 4)
    bias[1, :, 0] = np.tile(bf[1], 4)
    bias[2, :, 0] = np.tile(bf[2], 2)
    bias[3, :, 0] = np.tile(bf[3], 2)
    bias[4, :, 0] = bf[4]
    bias[5, :, 0] = np.asarray(gf_b, np.float64)
    d["biases"] = nf32(bias[:, :, 0].T)  # [128, 6]

    gfh = np.asarray(gf_h, np.float64)                   # [F, 3, F]
    d["hkT"] = nf32(np.concatenate([gfh[:, k, :].T for k in range(3)], axis=1))  # [128, 384]
    a0 = np.asarray(act_ws[0], np.float64)
    d["w1a"], d["w1b"] = nf32(a0[:128]), nf32(a0[128:])
    d["w2m"] = nf32(act_ws[1])
    d["w3m"] = nf32(act_ws[2])
    mb = np.zeros((2, 128, 1), np.float64)
    mb[0, :, 0] = np.asarray(act_bs[0], np.float64)
    mb[1, :, 0] = np.asarray(act_bs[1], np.float64)
    d["mbias"] = nf32(mb[:, :, 0].T)  # [128, 2]
    d["b3m"] = nf32(np.asarray(act_bs[2])[:, None])      # [2, 1]
    d["id20"] = nf32(np.eye(20))
    return d


def _prep_xs2d(x):  # [B, N, 100, 100] -> [B*N, 4, 54, 52] zero-padded bf16
    BN = x.shape[0] * x.shape[1]
    xr = np.asarray(x, np.float32).reshape(BN, 50, 2, 50, 2)
    out = np.zeros((BN, 4, 54, 52), np.float32)
    for py in range(2):
        for px in range(2):
            out[:, py * 2 + px, 2:52, 1:51] = xr[:, :, py, :, px]
    return out.astype(ml_dtypes.bfloat16)


# ---------------------------------------------------------------- device program
def _emit(nc, st):
    P = {}
    def par(name, shape, dt=BF16, out=False):
        P[name] = nc.declare_dram_parameter(name, list(shape), dt, isOutput=out)

    par("xs", (IPC, 4, 54, 52))
    par("w0s", (12, 384)); par("w1s", (96, 96)); par("w2s", (96, 192))
    par("w3a", (128, 192)); par("w3b", (64, 192))
    par("w4a", (128, 384)); par("w4b", (64, 384))
    par("compw", (144, 128, 128))
    par("compb", (1, 128), F32); par("ones1", (1, IPC), F32)
    par("biases", (128, 6), F32)
    par("Ssb", (20, SPC * 20), F32); par("hkT", (128, 384), F32)
    par("w1a", (128, 128), F32); par("w1b", (20, 128), F32)
    par("w2m", (128, 128), F32); par("w3m", (128, 2), F32)
    par("extras", (20, SPC * 20), F32); par("id20", (20, 20), F32)
    par("mbias", (128, 2), F32); par("b3m", (2, 1), F32)
    par("out", (SPC, 20, 2), F32, out=True)
    if DEBUG:
        par("dbg_feat", (IPC, 128), F32, out=True)
        par("dbg_pool0", (96, 2704), BF16, out=True)
        par("dbg_y1", (128, 2704), BF16, out=True)
        par("dbg_pool2", (128, 625), BF16, out=True)
        par("dbg_hall", (128, 144), F32, out=True)

    tc = st.enter_context(tile.TileContext(nc))
    pers = st.enter_context(tc.tile_pool(name="pers", bufs=1))
    sb = st.enter_context(tc.tile_pool(name="sb", bufs=2))
    sbt = st.enter_context(tc.tile_pool(name="sbt", bufs=6))
    cps = st.enter_context(tc.tile_pool(name="cps", bufs=5, space="PSUM"))
    tps = st.enter_context(tc.tile_pool(name="tps", bufs=2, space="PSUM"))
    cwp = st.enter_context(tc.tile_pool(name="cwp", bufs=8))

    def load(name, shape, dt=BF16, src=None, tag=None):
        t = pers.tile(list(shape), dt, tag=tag or name)
        nc.sync.dma_start(t[:], (src if src is not None else P[name])[:])
        return t

    w0s = load("w0s", (12, 384))
    w1s = load("w1s", (96, 96))
    w2s = load("w2s", (96, 192))
    w3a = load("w3a", (128, 192))
    w3b = load("w3b", (64, 192))
    w4a = load("w4a", (128, 384))
    w4b = load("w4b", (64, 384))
    biases = load("biases", (128, 6), F32)
    compb = load("compb", (1, 128), F32)
    ones1 = load("ones1", (1, IPC), F32)
    hall = pers.tile([128, IPC * 144], BF16, tag="hall")

    WS = {
        0: lambda dx: w0s[:, dx * 128:(dx + 1) * 128],
        1: lambda dx: w1s[:, dx * 32:(dx + 1) * 32],
        2: lambda dx: w2s[:, dx * 64:(dx + 1) * 64],
        "3a": lambda dx: w3a[:, dx * 64:(dx + 1) * 64],
        "3b": lambda dx: w3b[:, dx * 64:(dx + 1) * 64],
        "4a": lambda dx: w4a[:, dx * 128:(dx + 1) * 128],
        "4b": lambda dx: w4b[:, dx * 128:(dx + 1) * 128],
    }
    BIA = lambda i: biases[:, i:i + 1]

    def mkstacks(tag, n, p, f):
        ts = [pers.tile([p, f], BF16, tag=f"{tag}{i}", name=f"{tag}{i}")
              for i in range(n)]
        for t in ts:
            nc.gpsimd.memset(t[:], 0.0)
        return ts

    st1 = mkstacks("st1", 8, 96, 2704)     # slot = (gang%2)*4 + im%4
    st2 = mkstacks("st2", 4, 96, 2704)     # slot = ((im//2)%2)*2 + im%2
    st3a = mkstacks("st3a", 4, 128, 729)   # slot = ((im//2)%2)*2 + im%2
    st3b = mkstacks("st3b", 4, 64, 729)
    st4a = mkstacks("st4a", 4, 128, 729)   # slot = im%4
    st4b = mkstacks("st4b", 4, 64, 729)

    CH52 = [(r, min(8, 51 - r)) for r in range(1, 51, 8)]
    CH27 = [(1, 18), (19, 7)]

    def i3(t, p0, pn, r0, nr, c0, ncol, rs):
        return t[:].rearrange("p (r c) -> p r c", c=rs)[p0:p0 + pn,
                                                        r0:r0 + nr, c0:c0 + ncol]

    # ------------------------------------------------ L0 + pool0 -> st1
    def do_L0(im):
        stk = sb.tile([12, 52 * 52], BF16, tag="l0stk")
        for byi, sy in enumerate((-1, 0, 1)):
            nc.sync.dma_start(
                stk[:].rearrange("p (r c) -> p r c", c=52)[4 * byi:4 * byi + 4],
                P["xs"][im, :, 1 + sy:53 + sy, :])
        y = sb.tile([128, 2704], BF16, tag="l0y")
        for r, nr in CH52:
            pt = cps.tile([128, 416], F32, tag="cps")
            for dxi in range(3):
                nc.tensor.matmul(pt[:, :nr * 52], WS[0](dxi),
                                 stk[:, r * 52 + dxi - 1:(r + nr) * 52 + dxi - 1],
                                 start=(dxi == 0), stop=(dxi == 2))
            nc.scalar.activation(y[:, r * 52:(r + nr) * 52], pt[:, :nr * 52],
                                 AF.Lrelu, bias=BIA(0), alpha=LEAK)
        z1 = sbt.tile([32, 2704], BF16, tag="l0tmp")
        z2 = sbt.tile([32, 2704], BF16, tag="l0tmp")
        z3 = sbt.tile([32, 2704], BF16, tag="l0tmp")
        nc.sync.dma_start(z1[:, 52:2652], y[32:64, 52:2652])
        nc.sync.dma_start(z2[:, 52:2652], y[64:96, 52:2652])
        nc.sync.dma_start(z3[:, 52:2652], y[96:128, 52:2652])
        t1 = sbt.tile([32, 2704], BF16, tag="l0tmp")
        t2 = sbt.tile([32, 2704], BF16, tag="l0tmp")
        nc.vector.tensor_tensor(t1[:, 52:2652], y[0:32, 52:2652],
                                z1[:, 52:2652], op=ALU.max)
        nc.vector.tensor_tensor(t2[:, 52:2652], z2[:, 52:2652],
                                z3[:, 52:2652], op=ALU.max)
        dst = st1[((im // 4) % 2) * 4 + im % 4]
        nc.vector.tensor_tensor(i3(dst, 0, 32, 1, 50, 1, 50, 52),
                                i3(t1, 0, 32, 1, 50, 1, 50, 52),
                                i3(t2, 0, 32, 1, 50, 1, 50, 52), op=ALU.max)
        nc.sync.dma_start(i3(dst, 32, 32, 2, 50, 1, 50, 52),
                          i3(dst, 0, 32, 1, 50, 1, 50, 52))
        nc.sync.dma_start(i3(dst, 64, 32, 0, 50, 1, 50, 52),
                          i3(dst, 0, 32, 1, 50, 1, 50, 52))
        if DEBUG and im == 0:
            nc.sync.dma_start(P["dbg_pool0"][:], dst[:])

    # ------------------------------------------------ L1 (gang of 4) -> st2
    def do_L1(g):
        stks = [st1[(g % 2) * 4 + i] for i in range(4)]
        y = sb.tile([128, 2704], BF16, tag="l1y")
        for r, nr in CH52:
            pt = cps.tile([128, 416], F32, tag="cps")
            for i in range(4):
                for dxi in range(3):
                    nc.tensor.matmul(pt[32 * i:32 * i + 32, :nr * 52], WS[1](dxi),
                                     stks[i][:, r * 52 + dxi - 1:(r + nr) * 52 + dxi - 1],
                                     start=(dxi == 0), stop=(dxi == 2),
                                     tile_position=(0, 32 * i))
            nc.scalar.activation(y[:, r * 52:(r + nr) * 52], pt[:, :nr * 52],
                                 AF.Lrelu, bias=BIA(1), alpha=LEAK)
        if DEBUG and g == 0:
            nc.sync.dma_start(P["dbg_y1"][:], y[:])
        for i in range(4):
            im = 4 * g + i
            dst = st2[((im // 2) % 2) * 2 + im % 2]
            src = i3(y, 32 * i, 32, 1, 50, 1, 50, 52)
            nc.sync.dma_start(i3(dst, 0, 32, 1, 50, 1, 50, 52), src)
            nc.sync.dma_start(i3(dst, 32, 32, 2, 50, 1, 50, 52), src)
            nc.sync.dma_start(i3(dst, 64, 32, 0, 50, 1, 50, 52), src)

    # ------------------------------------------------ L2 (gang of 2) + pool -> st3
    def do_L2(g):
        stks = [st2[(g % 2) * 2 + i] for i in range(2)]
        y = sb.tile([128, 2704], BF16, tag="l2y")
        for r, nr in CH52:
            pt = cps.tile([128, 416], F32, tag="cps")
            for i in range(2):
                for dxi in range(3):
                    nc.tensor.matmul(pt[64 * i:64 * i + 64, :nr * 52], WS[2](dxi),
                                     stks[i][:, r * 52 + dxi - 1:(r + nr) * 52 + dxi - 1],
                                     start=(dxi == 0), stop=(dxi == 2),
                                     tile_position=(0, 64 * i))
            nc.scalar.activation(y[:, r * 52:(r + nr) * 52], pt[:, :nr * 52],
                                 AF.Lrelu, bias=BIA(2), alpha=LEAK)
        p1 = sb.tile([128, 1300], BF16, tag="l2p1")
        yv = y[:].rearrange("p (r c) -> p r c", c=52)
        nc.vector.tensor_tensor(p1[:].rearrange("p (r c) -> p r c", c=52),
                                yv[:, 1:51:2, :], yv[:, 2:52:2, :], op=ALU.max)
        p2 = sb.tile([128, 625], BF16, tag="l2p2")
        p1v = p1[:].rearrange("p (r c) -> p r c", c=52)
        nc.vector.tensor_tensor(p2[:].rearrange("p (r c) -> p r c", c=25),
                                p1v[:, :, 1:51:2], p1v[:, :, 2:52:2], op=ALU.max)
        if DEBUG and g == 0:
            nc.sync.dma_start(P["dbg_pool2"][:], p2[:])
        for i in range(2):
            im = 2 * g + i
            sl = ((im // 2) % 2) * 2 + im % 2
            src = p2[:].rearrange("p (r c) -> p r c", c=25)[64 * i:64 * i + 64]
            nc.sync.dma_start(i3(st3a[sl], 0, 64, 1, 25, 1, 25, 27), src)
            nc.sync.dma_start(i3(st3a[sl], 64, 64, 2, 25, 1, 25, 27), src)
            nc.sync.dma_start(i3(st3b[sl], 0, 64, 0, 25, 1, 25, 27), src)

    # ------------------------------------------------ L3 (gang of 2) -> st4
    def do_L3(g):
        y = sb.tile([128, 729], BF16, tag="l3y")
        for r, nr in CH27:
            pt = cps.tile([128, 486], F32, tag="cps")
            for i in range(2):
                sl = (g % 2) * 2 + i
                for dxi in range(3):
                    nc.tensor.matmul(pt[64 * i:64 * i + 64, :nr * 27], WS["3a"](dxi),
                                     st3a[sl][:, r * 27 + dxi - 1:(r + nr) * 27 + dxi - 1],
                                     start=(dxi == 0), stop=False,
                                     tile_position=(0, 64 * i))
                for dxi in range(3):
                    nc.tensor.matmul(pt[64 * i:64 * i + 64, :nr * 27], WS["3b"](dxi),
                                     st3b[sl][:, r * 27 + dxi - 1:(r + nr) * 27 + dxi - 1],
                                     start=False, stop=(dxi == 2),
                                     tile_position=(0, 64 * i))
            nc.scalar.activation(y[:, r * 27:(r + nr) * 27], pt[:, :nr * 27],
                                 AF.Lrelu, bias=BIA(3), alpha=LEAK)
        for i in range(2):
            im = 2 * g + i
            src = i3(y, 64 * i, 64, 1, 25, 1, 25, 27)
            nc.sync.dma_start(i3(st4a[im % 4], 0, 64, 1, 25, 1, 25, 27), src)
            nc.sync.dma_start(i3(st4a[im % 4], 64, 64, 2, 25, 1, 25, 27), src)
            nc.sync.dma_start(i3(st4b[im % 4], 0, 64, 0, 25, 1, 25, 27), src)

    # ------------------------------------------------ L4 + pool -> hall
    def do_L4(im):
        y = sb.tile([128, 729], BF16, tag="l4y")
        for r, nr in CH27:
            pt = cps.tile([128, 486], F32, tag="cps")
            for dxi in range(3):
                nc.tensor.matmul(pt[:, :nr * 27], WS["4a"](dxi),
                                 st4a[im % 4][:, r * 27 + dxi - 1:(r + nr) * 27 + dxi - 1],
                                 start=(dxi == 0), stop=False)
            for dxi in range(3):
                nc.tensor.matmul(pt[:, :nr * 27], WS["4b"](dxi),
                                 st4b[im % 4][:, r * 27 + dxi - 1:(r + nr) * 27 + dxi - 1],
                                 start=False, stop=(dxi == 2))
            nc.scalar.activation(y[:, r * 27:(r + nr) * 27], pt[:, :nr * 27],
                                 AF.Lrelu, bias=BIA(4), alpha=LEAK)
        p1 = sb.tile([128, 324], BF16, tag="l4p1")
        yv = y[:].rearrange("p (r c) -> p r c", c=27)
        nc.vector.tensor_tensor(p1[:].rearrange("p (r c) -> p r c", c=27),
                                yv[:, 1:25:2, :], yv[:, 2:26:2, :], op=ALU.max)
        p1v = p1[:].rearrange("p (r c) -> p r c", c=27)
        nc.vector.tensor_tensor(
            hall[:, im * 144:(im + 1) * 144].rearrange("p (r c) -> p r c", c=12),
            p1v[:, :, 1:25:2], p1v[:, :, 2:26:2], op=ALU.max)
        if DEBUG and im == 0:
            dh = sb.tile([128, 144], F32, tag="dbgh")
            nc.vector.tensor_copy(dh[:], hall[:, 0:144])
            nc.sync.dma_start(P["dbg_hall"][:], dh[:])

    # pipelined emission
    for g in range(NGANG):
        for i in range(4):
            do_L0(4 * g + i)
        do_L1(g)
        for h in (2 * g, 2 * g + 1):
            do_L2(h)
            do_L3(h)
        for i in range(4):
            do_L4(4 * g + i)

    # ------------------------------------------------ compress
    fp = tps.tile([IPC, 128], F32, tag="tps")
    for p in range(144):
        cw = cwp.tile([128, 128], BF16, tag="cwt")
        nc.sync.dma_start(cw[:], P["compw"][p])
        nc.tensor.matmul(fp[:], hall[:, p:p + 144 * (IPC - 1) + 1:144], cw[:],
                         start=(p == 0), stop=False)
    nc.tensor.matmul(fp[:], ones1[:], compb[:], start=False, stop=True)
    fsb = pers.tile([IPC, 128], F32, tag="fsb")
    nc.scalar.activation(fsb[:], fp[:], AF.Lrelu, alpha=LEAK)
    if DEBUG:
        nc.sync.dma_start(P["dbg_feat"][:], fsb[:])

    # ------------------------------------------------ tail (fp32)
    id20 = load("id20", (20, 20), F32)
    hkT = load("hkT", (128, 384), F32)
    w1a = load("w1a", (128, 128), F32)
    w1b = load("w1b", (20, 128), F32)
    w2m = load("w2m", (128, 128), F32)
    w3m = load("w3m", (128, 2), F32)
    mbias = load("mbias", (128, 2), F32)
    b3m = load("b3m", (2, 1), F32)
    Ss = load("Ssb", (20, SPC * 20), F32)
    exs = load("extras", (20, SPC * 20), F32)

    for s in range(SPC):
        w0 = pers.tile([20, 128], F32, tag=f"w0_{s}")
        if s == 0:
            nc.vector.tensor_copy(w0[:], fsb[0:20, :])
        else:
            nc.sync.dma_start(w0[:], fsb[20 * s:20 * s + 20, :])
        Sb = Ss[:, 20 * s:20 * s + 20]

        z0p = tps.tile([128, 20], F32, tag="tps")
        nc.tensor.transpose(z0p[:], w0[:], id20[:])
        z0 = pers.tile([128, 20], F32, tag=f"z0_{s}")
        nc.vector.tensor_copy(z0[:], z0p[:])

        w1p = tps.tile([20, 128], F32, tag="tps")
        nc.tensor.matmul(w1p[:], Sb, w0[:], start=True, stop=True)
        w1t = pers.tile([20, 128], F32, tag=f"w1t_{s}")
        nc.vector.tensor_copy(w1t[:], w1p[:])

        z1p = tps.tile([128, 20], F32, tag="tps")
        nc.tensor.matmul(z1p[:], w0[:], Sb, start=True, stop=True)
        z1 = pers.tile([128, 20], F32, tag=f"z1_{s}")
        nc.vector.tensor_copy(z1[:], z1p[:])

        z2p = tps.tile([128, 20], F32, tag="tps")
        nc.tensor.matmul(z2p[:], w1t[:], Sb, start=True, stop=True)
        z2 = pers.tile([128, 20], F32, tag=f"z2_{s}")
        nc.vector.tensor_copy(z2[:], z2p[:])

        yp = tps.tile([128, 20], F32, tag="tps")
        for k, zk in enumerate((z0, z1, z2)):
            nc.tensor.matmul(yp[:], hkT[:, 128 * k:128 * (k + 1)], zk[:],
                             start=(k == 0), stop=(k == 2))
        sfh = pers.tile([128, 20], F32, tag=f"sfh_{s}")
        nc.scalar.activation(sfh[:], yp[:], AF.Lrelu, bias=BIA(5), alpha=LEAK)

        a1p = tps.tile([128, 20], F32, tag="tps")
        nc.tensor.matmul(a1p[:], w1a[:], sfh[:], start=True, stop=False)
        nc.tensor.matmul(a1p[:], w1b[:], exs[:, 20 * s:20 * s + 20],
                         start=False, stop=True)
        a1 = pers.tile([128, 20], F32, tag=f"a1_{s}")
        nc.scalar.activation(a1[:], a1p[:], AF.Lrelu, bias=mbias[:, 0:1], alpha=LEAK)

        a2p = tps.tile([128, 20], F32, tag="tps")
        nc.tensor.matmul(a2p[:], w2m[:], a1[:], start=True, stop=True)
        a2 = pers.tile([128, 20], F32, tag=f"a2_{s}")
        nc.scalar.activation(a2[:], a2p[:], AF.Lrelu, bias=mbias[:, 1:2], alpha=LEAK)

        a3p = tps.tile([2, 20], F32, tag="tps")
        nc.tensor.matmul(a3p[:], w3m[:], a2[:], start=True, stop=True)
        a3 = pers.tile([2, 20], F32, tag=f"a3_{s}")
        nc.scalar.activation(a3[:], a3p[:], AF.Identity, bias=b3m[:])
        nc.sync.dma_start(P["out"][s].rearrange("n c -> c n"), a3[:])


_CACHED = {}


def _get_module():
    if "nc" not in _CACHED:
        nc = bacc.Bacc("TRN2", target_bir_lowering=False)
        with ExitStack() as st:
            _emit(nc, st)
        nc.finalize()
        _CACHED["nc"] = nc
    return _CACHED["nc"]


def kernel(x, S, refs, alphas, conv_ws, conv_bs, bn_gammas, bn_betas, bn_means,
           bn_vars, comp_w, comp_b, gf_h, gf_b, act_ws, act_bs):
    wd = _prep_weights(conv_ws, conv_bs, bn_gammas, bn_betas, bn_means, bn_vars,
                       comp_w, comp_b, gf_h, gf_b, act_ws, act_bs)
    xs = _prep_xs2d(x)
    refs = np.asarray(refs, np.float32)
    alphas = np.asarray(alphas, np.float32)
    S = np.asarray(S, np.float32)
    ex = np.concatenate([np.repeat(refs.transpose(0, 2, 1), 10, axis=1),
                         np.repeat(alphas.transpose(0, 2, 1), 10, axis=1)], axis=1)

    in_maps = []
    for c in range(NCORES):
        m = dict(wd)
        m["xs"] = xs[c * IPC:(c + 1) * IPC]
        m["Ssb"] = nf32(np.concatenate(list(S[c * SPC:(c + 1) * SPC]), axis=1))
        m["extras"] = nf32(np.concatenate(list(ex[c * SPC:(c + 1) * SPC]), axis=1))
        in_maps.append(m)

    nc = _get_module()
    res = run_bass_kernel_spmd(nc, in_maps, list(range(NCORES)))
    kernel.last_results = res.results
    out = np.concatenate([res.results[c]["out"] for c in range(NCORES)], axis=0)
    return out.reshape(B, N, 2).astype(np.float32)
